# revision 56
# baseline (speedup 1.0000x reference)
"""GCN/GAT 4-layer GNN on 8 Trainium2 NeuronCores.

Strategy (vertex-cut data parallelism), v2:
  - dst-nodes sharded 8 ways (6250/core); each core owns all edges into its
    shard (host graph partitioning, dst-sorted, grouped into 128-dst blocks).
  - Dense per-node matmuls on the owner core; per-layer gather tables
    (node features + src-side attention logits) AllGathered in TWO halves
    (split at local row 3200 = 25 blocks) so the first half of the collective
    overlaps the previous propagation tail. Local-row-half tables also keep
    every gather index within int16 range without a global src split.
  - Per-edge message passing: dma_gather fetches 128-edge chunks of src rows
    into SBUF; a {0,1} one-hot (edge -> dst_local) built on DVE turns the
    segmented sum into PSUM-accumulated matmuls. Gathers are round-robined
    over 4 SWDGE queues (descriptor generation parallelizes across Q7 core
    pairs); trailing padding indices are -1 so the ucode trims them.
  - GAT dst-side logits (ed) are never gathered: ed stays resident per block
    ([128, NB, 8]); a transposed one-hot (built from a partition-broadcast
    of dst_local via SBUF->SBUF DMA + one DVE compare) turns the per-edge
    ed fetch into a tiny PSUM matmul. exp weights are written back into the
    gathered rows' es slot so one fused matmul per chunk produces both the
    aggregate and the softmax denominator.
  - GAT softmax: exp without max-subtraction (logits are O(1)); GCN norm:
    dis[src] folded into table rows, dis[dst] as postscale (scalar engine).
  - LayerNorm mostly on the scalar engine (per-partition bias/scale APs).
"""

import sys, os
for _p in ("/opt/trn_rl_repo", "/root/.axon_site/_ro/trn_rl_repo"):
    if os.path.isdir(_p) and _p not in sys.path:
        sys.path.insert(0, _p)

import numpy as np
import ml_dtypes

import concourse.bass as bass
import concourse.bacc as bacc
import concourse.mybir as mybir
import concourse.tile as tile
from concourse.bass_utils import run_bass_kernel_spmd

F32 = mybir.dt.float32
BF16 = mybir.dt.bfloat16
I16 = mybir.dt.int16
BF = ml_dtypes.bfloat16
AF = mybir.ActivationFunctionType
OP = mybir.AluOpType

P = 8          # cores
HEADS = 8
NEG_SLOPE = 0.2
NUM_Q = int(os.environ.get("GNN_NUM_Q", "4"))  # SWDGE queues for gathers

# per-layer: row elems (gather granule, 256B-multiple), C = value cols,
# es at cols C..C+8 for GAT layers.
LAYERS = [
    dict(row=384, C=256, ch=32, gat=True),
    dict(row=128, C=128, ch=None, gat=False),
    dict(row=640, C=512, ch=64, gat=True),
    dict(row=128, C=64, ch=None, gat=False),
]


def _wrap_idx(idx, cap):
    """dma_gather idx layout: [128, cap/16] i16; idx i at (i%16, i//16),
    replicated 8x down partitions. Positions >= len(idx) gather row 0."""
    n = len(idx)
    cols = cap // 16
    arr = np.zeros((16, cols), np.int16)
    for off in range(16):
        sub = idx[off::16]
        arr[off, : len(sub)] = sub
    return np.tile(arr, (8, 1))


def preprocess(edge_index, N):
    """Graph partitioning + per-core index metadata with local-row-half
    tables. Returns (sched, percore)."""
    NSH = N // P
    NB = (NSH + 127) // 128
    NBA = min((NB + 1) // 2, NB)
    HA = min(NSH, NBA * 128)          # rows in half A (blocks 0..NBA-1)
    HB = NSH - HA

    src = np.concatenate([np.asarray(edge_index[0]), np.arange(N)]).astype(np.int64)
    dst = np.concatenate([np.asarray(edge_index[1]), np.arange(N)]).astype(np.int64)
    deg = np.bincount(dst, minlength=N).astype(np.float64)
    dis = (1.0 / np.sqrt(np.maximum(deg, 1e-12))).astype(np.float32)

    core = dst // NSH
    blk = (dst % NSH) // 128
    rsrc = src % NSH
    csrc = src // NSH
    hi = (rsrc >= HA).astype(np.int64)
    gidx = np.where(hi == 0, csrc * HA + rsrc, csrc * HB + (rsrc - HA))

    counts = np.zeros((P, NB, 2), np.int64)
    np.add.at(counts, (core, blk, hi), 1)
    K = np.maximum(1, -(-counts // 128)).max(axis=0)  # [NB, 2] chunks per half

    order = np.lexsort((hi, blk, core))
    so_g, so_dst, so_core, so_blk, so_hi = (a[order] for a in (gidx, dst, core, blk, hi))

    # block pairs (within each half) share one gather call per table side;
    # slot layout per pair: [A(b0) | A(b1) | B(b0) | B(b1)]
    def mk_pairs(lo, hi_):
        ps = []
        b = lo
        while b < hi_:
            ps.append(tuple(range(b, min(b + 2, hi_))))
            b += 2
        return ps

    pairs = mk_pairs(0, NBA) + mk_pairs(NBA, NB)
    tot_ch = int(K.sum())
    # chunk offsets: oA[pair], oB[pair]; groups emitted pair by pair
    oA, oB = {}, {}
    pos_ch = 0
    emit = []  # (block, half) emission order
    for pr in pairs:
        oA[pr] = pos_ch
        for b in pr:
            emit.append((b, 0))
            pos_ch += int(K[b, 0])
        oB[pr] = pos_ch
        for b in pr:
            emit.append((b, 1))
            pos_ch += int(K[b, 1])
    assert pos_ch == tot_ch

    percore = []
    for c in range(P):
        m = so_core == c
        cg, cdst = so_g[m], so_dst[m]
        # per-(block, half) edge slices within this core's sorted list
        ofs = {}
        p0 = 0
        for b in range(NB):
            for h in range(2):
                n_e = int(counts[c, b, h])
                ofs[(b, h)] = (p0, n_e)
                p0 += n_e
        assert p0 == m.sum()
        sidx = np.zeros(tot_ch * 128, np.int64)
        dloc = np.full((tot_ch * 128,), 200.0, np.float32)
        pos = 0
        sw = []
        for b, h in emit:
            p0, n_e = ofs[(b, h)]
            cap = int(K[b, h]) * 128
            sidx[pos: pos + n_e] = cg[p0: p0 + n_e]
            dloc[pos: pos + n_e] = (cdst[p0: p0 + n_e] - c * NSH - b * 128).astype(np.float32)
            sw.append(_wrap_idx(sidx[pos: pos + n_e], cap))
            pos += cap
        percore.append(dict(
            sidx=np.concatenate(sw, axis=1).astype(np.int16),
            dstl=dloc.reshape(tot_ch, 128).T.astype(BF).copy(),    # [128, tot_ch]
            dstlf=dloc.reshape(tot_ch, 128).astype(BF).copy(),     # [tot_ch, 128]
            dis=dis[c * NSH: (c + 1) * NSH],
        ))
    sched = dict(N=N, NSH=NSH, NB=NB, NBA=NBA, HA=HA, HB=HB, K=K, tot_ch=tot_ch,
                 pairs=pairs, oA=oA, oB=oB)
    return sched, percore


def prep_weights(w, N):
    """Host-side weight packing (bf16) shared by all cores."""
    def b16(a):
        return np.asarray(a, np.float32).astype(BF)

    W1 = np.asarray(w["g1_W"], np.float32)
    ws1s = np.einsum("khj,hj->kh", W1.reshape(1024, 8, 32), np.asarray(w["g1_as"], np.float32))
    ws1d = np.einsum("khj,hj->kh", W1.reshape(1024, 8, 32), np.asarray(w["g1_ad"], np.float32))
    w1cat = np.concatenate([W1, np.asarray(w["m1_W"], np.float32), ws1s, ws1d], axis=1)  # [1024, 528]
    w1t = b16(w1cat).reshape(8, 128, 528).transpose(1, 0, 2).copy()  # [128, 8, 528]

    w2cat = np.concatenate([np.asarray(w["g2_W"], np.float32), np.asarray(w["m2_W"], np.float32)], axis=1)
    w2t = b16(w2cat).reshape(2, 128, 256).transpose(1, 0, 2).copy()  # [128, 2, 256]

    W3 = np.asarray(w["g3_W"], np.float32)
    ws3s = np.einsum("khj,hj->kh", W3.reshape(128, 8, 64), np.asarray(w["g3_as"], np.float32))
    ws3d = np.einsum("khj,hj->kh", W3.reshape(128, 8, 64), np.asarray(w["g3_ad"], np.float32))
    w3t = b16(np.concatenate([W3, np.asarray(w["m3_W"], np.float32), ws3s, ws3d], axis=1))  # [128, 592]

    w4t = b16(np.concatenate([np.asarray(w["g4_W"], np.float32), np.asarray(w["m4_W"], np.float32)], axis=1))  # [64, 4]

    def rep(v):
        return np.tile(np.asarray(v, np.float32)[None, :], (128, 1)).copy()

    return dict(
        w1=w1t, w2=w2t, w3=w3t, w4=w4t,
        bias1=rep(np.asarray(w["g1_b"]) + np.asarray(w["m1_b"])),
        bias2=rep(np.asarray(w["g2_b"]) + np.asarray(w["m2_b"])),
        bias3=rep(np.asarray(w["g3_b"]) + np.asarray(w["m3_b"])),
        bias4=rep(np.asarray(w["g4_b"]) + np.asarray(w["m4_b"])),
        iota=np.tile(np.arange(128, dtype=np.float32)[None, :], (128, 1)).astype(BF),
        iotap=np.arange(128, dtype=np.float32)[:, None].astype(BF).copy(),
        ident=np.eye(128, dtype=np.float32).astype(BF),
        onesr=np.ones((1, 128), dtype=np.float32).astype(BF),
        b1r=np.asarray(w["g1_b"] + w["m1_b"], np.float32)[None, :].astype(BF),
        b2r=np.asarray(w["g2_b"] + w["m2_b"], np.float32)[None, :].astype(BF),
        b3r=np.asarray(w["g3_b"] + w["m3_b"], np.float32)[None, :].astype(BF),
    )


def bcast(ap, count):
    """Append a 0-stride broadcast dim of `count` to an AP."""
    return bass.AP(ap.tensor, ap.offset, list(ap.ap) + [[0, count]])


def mid_bcast(ap, count):
    """Insert a 0-stride dim of `count` after the partition dim."""
    a = list(ap.ap)
    return bass.AP(ap.tensor, ap.offset, [a[0], [0, count]] + a[1:])


def flat_view(t_ap, c0, rowstride, a, b, nch=None):
    """View chunk-major flat tile [128, W] as rows: [128, nch, b-a] starting
    at chunk c0, row stride `rowstride`, cols [a, b)."""
    p0 = t_ap.ap[0]
    if nch is None:
        return bass.AP(t_ap.tensor, t_ap.offset + c0 * rowstride + a,
                       [p0, [1, b - a]])
    return bass.AP(t_ap.tensor, t_ap.offset + c0 * rowstride + a,
                   [p0, [rowstride, nch], [1, b - a]])


def build_nc(sched):
    N, NSH, NB, NBA = sched["N"], sched["NSH"], sched["NB"], sched["NBA"]
    HA, HB, K, tot_ch = sched["HA"], sched["HB"], sched["K"], sched["tot_ch"]
    pairs, oA, oB = sched["pairs"], sched["oA"], sched["oB"]
    pairs_proc = [p for p in pairs if p[0] >= NBA] + [p for p in pairs if p[0] < NBA]
    PKT = max(int(sum(K[b].sum() for b in pr)) for pr in pairs)
    ROWMAX = max(c["row"] for c in LAYERS)

    nc = bacc.Bacc("TRN2", target_bir_lowering=False, debug=False, num_devices=P,
                   num_swdge_queues=NUM_Q)

    # ---- I/O ----
    xT_in = nc.dram_tensor("xT", [NB, 128, 1024], BF16, kind="ExternalInput")
    w1_in = nc.dram_tensor("w1", [128, 8, 528], BF16, kind="ExternalInput")
    w2_in = nc.dram_tensor("w2", [128, 2, 256], BF16, kind="ExternalInput")
    w3_in = nc.dram_tensor("w3", [128, 592], BF16, kind="ExternalInput")
    w4_in = nc.dram_tensor("w4", [64, 4], BF16, kind="ExternalInput")
    b1_in = nc.dram_tensor("bias1", [128, 256], F32, kind="ExternalInput")
    b2_in = nc.dram_tensor("bias2", [128, 128], F32, kind="ExternalInput")
    b3_in = nc.dram_tensor("bias3", [128, 64], F32, kind="ExternalInput")
    b4_in = nc.dram_tensor("bias4", [128, 2], F32, kind="ExternalInput")
    iota_in = nc.dram_tensor("iota", [128, 128], BF16, kind="ExternalInput")
    iotap_in = nc.dram_tensor("iotap", [128, 1], BF16, kind="ExternalInput")
    id_in = nc.dram_tensor("ident", [128, 128], BF16, kind="ExternalInput")
    onesr_in = nc.dram_tensor("onesr", [1, 128], BF16, kind="ExternalInput")
    b1r_in = nc.dram_tensor("b1r", [1, 256], BF16, kind="ExternalInput")
    b2r_in = nc.dram_tensor("b2r", [1, 128], BF16, kind="ExternalInput")
    b3r_in = nc.dram_tensor("b3r", [1, 64], BF16, kind="ExternalInput")
    dis_in = nc.dram_tensor("dis", [128, NB], F32, kind="ExternalInput")
    sidx_in = nc.dram_tensor("sidx", [128, tot_ch * 8], I16, kind="ExternalInput")
    dstl_in = nc.dram_tensor("dstl", [128, tot_ch], BF16, kind="ExternalInput")
    dstlf_in = nc.dram_tensor("dstlf", [tot_ch, 128], BF16, kind="ExternalInput")
    out_dram = nc.dram_tensor("out", [NSH, 2], F32, kind="ExternalOutput")

    # ---- internal DRAM: bounce shards (two halves) + allgathered tables ----
    mlp0_dram = nc.dram_tensor("mlp0d", [NB, 128, 256], BF16)
    mlp1_dram = nc.dram_tensor("mlp1d", [NB, 128, 128], BF16)
    rows = [c["row"] for c in LAYERS]
    TbA = [nc.dram_tensor(f"T{l+1}bA", [HA, rows[l]], BF16) for l in range(4)]
    TbB = [nc.dram_tensor(f"T{l+1}bB", [HB, rows[l]], BF16) for l in range(4)]
    TgA = [nc.dram_tensor(f"T{l+1}A", [P * HA, rows[l]], BF16, addr_space="Shared")
           for l in range(4)]
    TgB = [nc.dram_tensor(f"T{l+1}B", [P * HB, rows[l]], BF16, addr_space="Shared")
           for l in range(4)]

    blk_rows = [min(128, NSH - b * 128) for b in range(NB)]
    koff = np.concatenate([[0], np.cumsum(K.sum(axis=1))]).astype(int)

    gq = [0]  # gather queue round-robin
    pref = {}  # per-block prefetch stash

    def next_q():
        q = gq[0]
        gq[0] = (q + 1) % NUM_Q
        return q

    with tile.TileContext(nc) as tc:
        with (
            tc.tile_pool(name="consts", bufs=1) as cpool,
            tc.tile_pool(name="resident", bufs=1) as rpool,
            tc.tile_pool(name="meta", bufs=1) as mpool,
            tc.tile_pool(name="xload", bufs=2) as xpool,
            tc.tile_pool(name="gath", bufs=2) as gpool,
            tc.tile_pool(name="onehot", bufs=2) as opool,
            tc.tile_pool(name="dstlp", bufs=1) as dpool,
            tc.tile_pool(name="work", bufs=2) as wpool,
            tc.tile_pool(name="asm", bufs=2) as apool,
            tc.tile_pool(name="psA", bufs=2, space="PSUM") as psA,
            tc.tile_pool(name="psS", bufs=2, space="PSUM") as psS,
            tc.tile_pool(name="psT", bufs=2, space="PSUM") as psT,
            tc.tile_pool(name="psE", bufs=2, space="PSUM") as psE,
        ):
            # ---------- constants ----------
            w1_t = cpool.tile([128, 8, 528], BF16); nc.sync.dma_start(w1_t[:], w1_in[:])
            w2_t = cpool.tile([128, 2, 256], BF16); nc.sync.dma_start(w2_t[:], w2_in[:])
            w3_t = cpool.tile([128, 592], BF16); nc.sync.dma_start(w3_t[:], w3_in[:])
            w4_t = cpool.tile([64, 4], BF16); nc.sync.dma_start(w4_t[:], w4_in[:])
            b1_t = cpool.tile([128, 256], F32); nc.sync.dma_start(b1_t[:], b1_in[:])
            b2_t = cpool.tile([128, 128], F32); nc.sync.dma_start(b2_t[:], b2_in[:])
            b3_t = cpool.tile([128, 64], F32); nc.sync.dma_start(b3_t[:], b3_in[:])
            b4_t = cpool.tile([128, 2], F32); nc.sync.dma_start(b4_t[:], b4_in[:])
            iota_t = cpool.tile([128, 128], BF16); nc.sync.dma_start(iota_t[:], iota_in[:])
            iop_t = cpool.tile([128, 1], BF16); nc.sync.dma_start(iop_t[:], iotap_in[:])
            id_t = cpool.tile([128, 128], BF16); nc.sync.dma_start(id_t[:], id_in[:])
            ones_t = cpool.tile([1, 128], BF16); nc.sync.dma_start(ones_t[:], onesr_in[:])
            b1r_t = cpool.tile([1, 256], BF16); nc.sync.dma_start(b1r_t[:], b1r_in[:])
            b2r_t = cpool.tile([1, 128], BF16); nc.sync.dma_start(b2r_t[:], b2r_in[:])
            b3r_t = cpool.tile([1, 64], BF16); nc.sync.dma_start(b3r_t[:], b3r_in[:])
            dis_t = cpool.tile([128, NB], F32); nc.sync.dma_start(dis_t[:], dis_in[:])
            zcol = cpool.tile([128, 1], F32); nc.vector.memset(zcol[:], 0.0)
            epscol = cpool.tile([128, 1], F32); nc.vector.memset(epscol[:], 1e-5)
            sidx_t = mpool.tile([128, tot_ch * 8], I16); nc.sync.dma_start(sidx_t[:], sidx_in[:])
            dstl_t = mpool.tile([128, tot_ch], BF16); nc.sync.dma_start(dstl_t[:], dstl_in[:])

            mlp2_sb = rpool.tile([128, NB, 64], BF16)
            x3_sb = rpool.tile([128, NB, 64], BF16)
            edl1 = rpool.tile([128, NB, 8], BF16)
            edl3 = rpool.tile([128, NB, 8], BF16)

            def tb_write(l, b, src_ap):
                r = blk_rows[b]
                if b < NBA:
                    nc.sync.dma_start(TbA[l][b * 128: b * 128 + r, :], src_ap[0:r, :])
                else:
                    r0 = b * 128 - HA
                    nc.sync.dma_start(TbB[l][r0: r0 + r, :], src_ap[0:r, :])

            def ag(l, half):
                src, dst = (TbA[l], TgA[l]) if half == 0 else (TbB[l], TgB[l])
                nc.gpsimd.collective_compute(
                    "AllGather", OP.bypass, replica_groups=[list(range(P))],
                    ins=[src.ap().opt()], outs=[dst.ap().opt()])

            # B-half blocks first so the B AllGather is emitted early and its
            # flight hides under the A-half processing of the same phase.
            order_blocks = list(range(NBA, NB)) + list(range(NBA))

            # ---------- dense phase 1: h1|mlp1|es1|ed1 from x ----------
            for b in order_blocks:
                xt = xpool.tile([128, 1024], BF16)
                nc.sync.dma_start(xt[:], xT_in[b])
                pd = psA.tile([128, 512], F32, tag="psA")
                pe = psS.tile([128, 16], F32, tag="psS")
                for k in range(8):
                    nc.tensor.matmul(pd[:], xt[:, k * 128:(k + 1) * 128],
                                     w1_t[:, k, 0:512], start=(k == 0), stop=(k == 7))
                nc.tensor.matmul(pd[:, 256:512], ones_t[:], b1r_t[:],
                                 start=False, stop=True, skip_group_check=True)
                for k in range(8):
                    nc.tensor.matmul(pe[:], xt[:, k * 128:(k + 1) * 128],
                                     w1_t[:, k, 512:528], start=(k == 0), stop=(k == 7))
                as1 = apool.tile([128, 384], BF16, tag="as1")
                nc.scalar.copy(as1[:, 0:256], pd[:, 0:256])
                nc.scalar.copy(as1[:, 256:264], pe[:, 0:8])
                tb_write(0, b, as1)
                nc.scalar.copy(edl1[:, b, :], pe[:, 8:16])
                m0 = apool.tile([128, 256], BF16, tag="m0")
                nc.scalar.copy(m0[:], pd[:, 256:512])
                nc.sync.dma_start(mlp0_dram[b], m0[:])
                if b == NB - 1:
                    ag(0, 1)
            ag(0, 0)

            # one-time zero of gather buffers (stale rows of padded gather
            # slots are read by DVE/PE; first use must not be NaN garbage)
            for _ in range(2):
                g = gpool.tile([128, PKT * ROWMAX], BF16, tag="G")
                nc.vector.memset(g[:], 0.0)

            # ---------- propagation (block PAIRS share one gather pass) ----
            def prop(l, epilogue):
                cfg = LAYERS[l - 1]
                row, C, gat = cfg["row"], cfg["C"], cfg["gat"]
                ch = cfg["ch"]
                edl = edl1 if l == 1 else (edl3 if l == 3 else None)
                for pr in pairs_proc:
                    o = int(oA[pr])
                    kas = [int(K[b, 0]) for b in pr]
                    kbs = [int(K[b, 1]) for b in pr]
                    lenA, lenB = sum(kas), sum(kbs)
                    ktp = lenA + lenB
                    blockof = [b for b, k in zip(pr, kas) for _ in range(k)] + \
                              [b for b, k in zip(pr, kbs) for _ in range(k)]
                    g = gpool.tile([128, PKT * ROWMAX], BF16, tag="G")
                    nc.gpsimd.dma_gather(
                        out_ap=flat_view(g[:], lenA, row, 0, row, lenB),
                        in_ap=TgB[l - 1][:],
                        idxs_ap=sidx_t[:, (o + lenA) * 8: (o + ktp) * 8],
                        num_idxs=lenB * 128, num_idxs_reg=lenB * 128, elem_size=row,
                        single_packet=False, queue_num=next_q())
                    nc.gpsimd.dma_gather(
                        out_ap=flat_view(g[:], 0, row, 0, row, lenA),
                        in_ap=TgA[l - 1][:],
                        idxs_ap=sidx_t[:, o * 8: (o + lenA) * 8],
                        num_idxs=lenA * 128, num_idxs_reg=lenA * 128, elem_size=row,
                        single_packet=False, queue_num=next_q())

                    def gview(c, a2, b2, n=None):
                        """row view of chunk c (pair-relative chunk id)."""
                        return flat_view(g[:], c, row, a2, b2, n)

                    if l == 1:
                        for b in pr:
                            m0p = wpool.tile([128, 256], BF16, tag="m0l")
                            nc.sync.dma_start(m0p[:], mlp0_dram[b])
                            pref[("m0", b)] = m0p
                    if l == 2:
                        for b in pr:
                            m1p = wpool.tile([128, 128], BF16, tag="m1l")
                            nc.sync.dma_start(m1p[:], mlp1_dram[b])
                            pref[("m1", b)] = m1p

                    # one-hot [e, (k, d)] = (iota[d] == dstl[k][e]) whole pair
                    oh = opool.tile([128, PKT * 128], BF16, tag="oh")
                    ohv = bass.AP(oh[:].tensor, oh[:].offset,
                                  [oh[:].ap[0], [128, ktp], [1, 128]])
                    nc.vector.tensor_tensor(
                        ohv, mid_bcast(iota_t[:], ktp),
                        bcast(dstl_t[:, o: o + ktp], 128), OP.is_equal)

                    if gat:
                        # transposed one-hot from a partition-broadcast of
                        # dst_local (per-edge ed fetch), whole pair at once
                        dstlP = dpool.tile([128, PKT * 128], BF16, tag="dstlP")
                        nc.sync.dma_start(dstlP[0:1, 0: ktp * 128],
                                          dstlf_in[o: o + ktp, :])
                        nc.gpsimd.partition_broadcast(
                            dstlP[:, 0: ktp * 128], dstlP[0:1, 0: ktp * 128])
                        ohT = dpool.tile([128, PKT * 128], BF16, tag="ohT")
                        dPv = bass.AP(dstlP[:].tensor, dstlP[:].offset,
                                      [dstlP[:].ap[0], [128, ktp], [1, 128]])
                        iopv = bass.AP(iop_t[:].tensor, iop_t[:].offset,
                                       [iop_t[:].ap[0], [0, ktp], [0, 128]])
                        ohTv = bass.AP(ohT[:].tensor, ohT[:].offset,
                                       [ohT[:].ap[0], [128, ktp], [1, 128]])
                        nc.vector.tensor_tensor(ohTv, dPv, iopv, OP.is_equal)
                        pse = psE.tile([128, PKT, 8], F32, tag="psE")
                        for c in range(ktp):
                            nc.tensor.matmul(pse[:, c, :],
                                             bass.AP(ohT[:].tensor, ohT[:].offset + c * 128,
                                                     [ohT[:].ap[0], [1, 128]]),
                                             edl[:, blockof[c], :], start=True, stop=True,
                                             skip_group_check=True)
                        # logits -> exp weights: exp(lrelu(z)) = max(e^z, e^.2z)
                        tsum = wpool.tile([128, PKT, 8], BF16, tag="tsum")
                        nc.vector.tensor_tensor(tsum[:, 0:ktp, :], gview(0, C, C + 8, ktp),
                                                pse[:, 0:ktp, :], OP.add)
                        e1 = wpool.tile([128, PKT, 8], BF16, tag="e1")
                        nc.scalar.activation(e1[:, 0:ktp, :], tsum[:, 0:ktp, :],
                                             AF.Exp, bias=zcol[:])
                        e5 = wpool.tile([128, PKT, 8], BF16, tag="e5")
                        nc.scalar.activation(e5[:, 0:ktp, :], tsum[:, 0:ktp, :],
                                             AF.Exp, bias=zcol[:], scale=NEG_SLOPE)
                        nc.vector.tensor_tensor(gview(0, C, C + 8, ktp), e1[:, 0:ktp, :],
                                                e5[:, 0:ktp, :], OP.max)
                        # sc = h * exp (in place, head-broadcast), whole pair
                        hview = gview(0, 0, C, ktp)
                        h4 = bass.AP(hview.tensor, hview.offset,
                                     [hview.ap[0], [row, ktp], [ch, HEADS], [1, ch]])
                        ex = gview(0, C, C + 8, ktp)
                        exb = bass.AP(ex.tensor, ex.offset,
                                      [ex.ap[0], [row, ktp], [1, HEADS], [0, ch]])
                        nc.vector.tensor_tensor(h4, h4, exb, OP.mult)

                    # per-block aggregation + epilogue
                    for j, b in enumerate(pr):
                        a0 = sum(kas[:j])
                        b0 = lenA + sum(kbs[:j])
                        corder = list(range(b0, b0 + kbs[j])) + \
                                 list(range(a0, a0 + kas[j]))
                        kt = len(corder)
                        pagg = psA.tile([128, 264] if gat else [128, C], F32, tag="psA")
                        if gat:
                            if C == 256:
                                for i, c in enumerate(corder):
                                    nc.tensor.matmul(pagg[:], ohv[:, c, :], gview(c, 0, 264),
                                                     start=(i == 0), stop=(i == kt - 1),
                                                     skip_group_check=True)
                                den = pagg[:, 256:264]
                                pag2 = None
                            else:
                                pag2 = psA.tile([128, 264], F32, tag="psA")
                                for i, c in enumerate(corder):
                                    nc.tensor.matmul(pagg[:, 0:256], ohv[:, c, :], gview(c, 0, 256),
                                                     start=(i == 0), stop=(i == kt - 1),
                                                     skip_group_check=True)
                                for i, c in enumerate(corder):
                                    nc.tensor.matmul(pag2[:], ohv[:, c, :], gview(c, 256, 520),
                                                     start=(i == 0), stop=(i == kt - 1),
                                                     skip_group_check=True)
                                den = pag2[:, 256:264]
                            sden = wpool.tile([128, 8], F32, tag="sden")
                            nc.vector.tensor_scalar(sden[:], den, 1e-16, None, OP.add)
                            rs = wpool.tile([128, 8], F32, tag="rs")
                            nc.vector.reciprocal(rs[:], sden[:])
                            agf = wpool.tile([128, C], F32, tag="agf")
                            if C == 256:
                                nc.vector.tensor_tensor(
                                    agf[:].rearrange("p (h j) -> p h j", j=ch),
                                    pagg[:, 0:256].rearrange("p (h j) -> p h j", j=ch),
                                    bcast(rs[:], ch), OP.mult)
                            else:
                                nc.vector.tensor_tensor(
                                    agf[:, 0:256].rearrange("p (h j) -> p h j", j=ch),
                                    pagg[:, 0:256].rearrange("p (h j) -> p h j", j=ch),
                                    bcast(rs[:, 0:4], ch), OP.mult)
                                nc.vector.tensor_tensor(
                                    agf[:, 256:512].rearrange("p (h j) -> p h j", j=ch),
                                    pag2[:, 0:256].rearrange("p (h j) -> p h j", j=ch),
                                    bcast(rs[:, 4:8], ch), OP.mult)
                            epilogue(b, agf)
                        else:
                            for i, c in enumerate(corder):
                                nc.tensor.matmul(pagg[:], ohv[:, c, :], gview(c, 0, C),
                                                 start=(i == 0), stop=(i == kt - 1),
                                                 skip_group_check=True)
                            agf = wpool.tile([128, C], F32, tag="agf")
                            nc.scalar.mul(agf[:], pagg[:], dis_t[:, b: b + 1])
                            epilogue(b, agf)
                    if NB - 1 in pr and l < 4:
                        ag(l, 1)
                if l < 4:
                    ag(l, 0)

            def layer_norm(t, Cn):
                """LN over free dim (ln weight=1 bias=0); scalar-engine heavy.
                In-place: t is centered and scaled; returns bf16 tile."""
                mu = wpool.tile([128, 1], F32, tag="mu")
                nc.vector.tensor_reduce(mu[:], t[:], mybir.AxisListType.X, OP.add)
                mun = wpool.tile([128, 1], F32, tag="mun")
                nc.scalar.mul(mun[:], mu[:], -1.0 / Cn)
                nc.scalar.activation(t[:], t[:], AF.Identity, bias=mun[:])
                sq = wpool.tile([128, Cn], BF16, tag="sq")
                vs = wpool.tile([128, 1], F32, tag="vs")
                nc.scalar.activation(sq[:], t[:], AF.Square, bias=zcol[:], accum_out=vs[:])
                sd = wpool.tile([128, 1], F32, tag="sd")
                nc.scalar.activation(sd[:], vs[:], AF.Sqrt, bias=epscol[:], scale=1.0 / Cn)
                rstd = wpool.tile([128, 1], F32, tag="rstd")
                nc.vector.reciprocal(rstd[:], sd[:])
                xo = wpool.tile([128, Cn], BF16, tag="xo")
                nc.scalar.mul(xo[:], t[:], rstd[:])
                return xo

            def transpose_to(x_ap, cols):
                pt = psT.tile([128, 128], BF16, tag="psT")
                nc.tensor.transpose(pt[0:cols, :], x_ap, id_t[:])
                xt_ = wpool.tile([cols, 128], BF16, tag=f"tr{cols}")
                nc.scalar.copy(xt_[:], pt[0:cols, :])
                return xt_

            # ---- L1 epilogue: LN -> x1, dense-2 (h2'|mlp2), T2 assembly ----
            def epi1(b, agf):
                nc.vector.tensor_tensor(agf[:], agf[:], pref.pop(("m0", b))[:], OP.add)
                x1 = layer_norm(agf, 256)
                xta = transpose_to(x1[:, 0:128], 128)
                xtb = transpose_to(x1[:, 128:256], 128)
                ps2 = psS.tile([128, 256], F32, tag="psS")
                nc.tensor.matmul(ps2[:], xta[:], w2_t[:, 0, :], start=True, stop=False)
                nc.tensor.matmul(ps2[:], xtb[:], w2_t[:, 1, :], start=False, stop=True)
                nc.tensor.matmul(ps2[:, 128:256], ones_t[:], b2r_t[:],
                                 start=False, stop=True, skip_group_check=True)
                as2 = apool.tile([128, 128], BF16, tag="as2")
                nc.scalar.mul(as2[:], ps2[:, 0:128], dis_t[:, b: b + 1])
                tb_write(1, b, as2)
                m1 = apool.tile([128, 128], BF16, tag="m1")
                nc.scalar.copy(m1[:], ps2[:, 128:256])
                nc.sync.dma_start(mlp1_dram[b], m1[:])

            # ---- L2 epilogue: LN -> x2, dense-3, T3 assembly ----
            def epi2(b, agf):
                nc.vector.tensor_tensor(agf[:], agf[:], pref.pop(("m1", b))[:], OP.add)
                x2 = layer_norm(agf, 128)
                xt2 = transpose_to(x2[:], 128)
                ps3a = psA.tile([128, 512], F32, tag="psA")
                nc.tensor.matmul(ps3a[:], xt2[:], w3_t[:, 0:512], start=True, stop=True)
                ps3b = psS.tile([128, 80], F32, tag="psS")
                nc.tensor.matmul(ps3b[:], xt2[:], w3_t[:, 512:592], start=True, stop=False)
                nc.tensor.matmul(ps3b[:, 0:64], ones_t[:], b3r_t[:],
                                 start=False, stop=True, skip_group_check=True)
                as3 = apool.tile([128, 640], BF16, tag="as3")
                nc.scalar.copy(as3[:, 0:512], ps3a[:])
                nc.scalar.copy(as3[:, 512:520], ps3b[:, 64:72])
                tb_write(2, b, as3)
                nc.scalar.copy(edl3[:, b, :], ps3b[:, 72:80])
                nc.scalar.copy(mlp2_sb[:, b, :], ps3b[:, 0:64])

            # ---- L3 epilogue: mean heads, LN -> x3, T4 assembly ----
            def epi3(b, agf):
                mf = wpool.tile([128, 64], F32, tag="mf")
                a = agf[:]
                nc.vector.tensor_reduce(
                    mf[:], bass.AP(a.tensor, a.offset, [a.ap[0], [1, 64], [64, 8]]),
                    mybir.AxisListType.X, OP.add)
                t = wpool.tile([128, 64], F32, tag="t3")
                nc.scalar.mul(t[:], mf[:], 0.125)
                nc.vector.tensor_tensor(t[:], t[:], mlp2_sb[:, b, :], OP.add)
                x3 = layer_norm(t, 64)
                nc.scalar.copy(x3_sb[:, b, :], x3[:])
                as4 = apool.tile([128, 128], BF16, tag="as4")
                nc.scalar.mul(as4[:, 0:64], x3[:], dis_t[:, b: b + 1])
                tb_write(3, b, as4)

            # ---- L4 epilogue: (agg @ W4) + (x3 @ m4_W) + bias ----
            def epi4(b, agf):
                a4 = wpool.tile([128, 64], BF16, tag="a4")
                nc.scalar.copy(a4[:], agf[:])
                a4T = transpose_to(a4[:], 64)
                x3T = transpose_to(x3_sb[:, b, :], 64)
                ps4 = psS.tile([128, 2], F32, tag="psS")
                nc.tensor.matmul(ps4[:], a4T[:], w4_t[:, 0:2], start=True, stop=False)
                nc.tensor.matmul(ps4[:], x3T[:], w4_t[:, 2:4], start=False, stop=True)
                ot = wpool.tile([128, 2], F32, tag="ot")
                nc.vector.tensor_tensor(ot[:], ps4[:], b4_t[:], OP.add)
                nc.sync.dma_start(out_dram[b * 128: b * 128 + blk_rows[b], :], ot[0:blk_rows[b], :])

            prop(1, epi1)
            prop(2, epi2)
            prop(3, epi3)
            prop(4, epi4)

    nc.compile()
    return nc


def make_in_maps(inputs, sched, percore):
    N, NSH, NB = sched["N"], sched["NSH"], sched["NB"]
    wm = prep_weights(inputs, N)
    x = np.asarray(inputs["x"], np.float32)
    in_maps = []
    for c in range(P):
        xs = x[c * NSH: (c + 1) * NSH]
        pad = NB * 128 - NSH
        if pad:
            xs = np.concatenate([xs, np.zeros((pad, 1024), np.float32)], 0)
        xT = xs.astype(BF).reshape(NB, 128, 8, 128).transpose(0, 3, 2, 1).reshape(NB, 128, 1024).copy()
        pc = percore[c]
        dis = np.zeros((128, NB), np.float32)
        dv = pc["dis"]
        for b in range(NB):
            r = min(128, NSH - b * 128)
            dis[0:r, b] = dv[b * 128: b * 128 + r]
        in_maps.append(dict(
            xT=xT, w1=wm["w1"], w2=wm["w2"], w3=wm["w3"], w4=wm["w4"],
            bias1=wm["bias1"], bias2=wm["bias2"], bias3=wm["bias3"], bias4=wm["bias4"],
            iota=wm["iota"], iotap=wm["iotap"], ident=wm["ident"], dis=dis,
            onesr=wm["onesr"], b1r=wm["b1r"], b2r=wm["b2r"], b3r=wm["b3r"],
            sidx=pc["sidx"], dstl=pc["dstl"], dstlf=pc["dstlf"],
        ))
    return in_maps


def run(inputs, N=50000, trace=False):
    sched, percore = preprocess(np.asarray(inputs["edge_index"]), N)
    in_maps = make_in_maps(inputs, sched, percore)
    nc = build_nc(sched)
    res = run_bass_kernel_spmd(nc, in_maps, core_ids=list(range(P)), trace=trace)
    out = np.concatenate([res.results[c]["out"] for c in range(P)], axis=0)
    return out, res


def kernel(**inputs):
    out, _ = run(inputs, N=50000)
    return out.astype(np.float32)


# revision 57
# speedup vs baseline: 1.3525x; 1.3525x over previous
"""GCN/GAT 4-layer GNN on 8 Trainium2 NeuronCores.

Strategy (vertex-cut data parallelism), v2:
  - dst-nodes sharded 8 ways (6250/core); each core owns all edges into its
    shard (host graph partitioning, dst-sorted, grouped into 128-dst blocks).
  - Dense per-node matmuls on the owner core; per-layer gather tables
    (node features + src-side attention logits) AllGathered in TWO halves
    (split at local row 3200 = 25 blocks) so the first half of the collective
    overlaps the previous propagation tail. Local-row-half tables also keep
    every gather index within int16 range without a global src split.
  - Per-edge message passing: dma_gather fetches 128-edge chunks of src rows
    into SBUF; a {0,1} one-hot (edge -> dst_local) built on DVE turns the
    segmented sum into PSUM-accumulated matmuls. Gathers are round-robined
    over 4 SWDGE queues (descriptor generation parallelizes across Q7 core
    pairs); trailing padding indices are -1 so the ucode trims them.
  - GAT dst-side logits (ed) are never gathered: ed stays resident per block
    ([128, NB, 8]); a transposed one-hot (built from a partition-broadcast
    of dst_local via SBUF->SBUF DMA + one DVE compare) turns the per-edge
    ed fetch into a tiny PSUM matmul. exp weights are written back into the
    gathered rows' es slot so one fused matmul per chunk produces both the
    aggregate and the softmax denominator.
  - GAT softmax: exp without max-subtraction (logits are O(1)); GCN norm:
    dis[src] folded into table rows, dis[dst] as postscale (scalar engine).
  - LayerNorm mostly on the scalar engine (per-partition bias/scale APs).
"""

import sys, os
for _p in ("/opt/trn_rl_repo", "/root/.axon_site/_ro/trn_rl_repo"):
    if os.path.isdir(_p) and _p not in sys.path:
        sys.path.insert(0, _p)

import numpy as np
import ml_dtypes

import concourse.bass as bass
import concourse.bacc as bacc
import concourse.mybir as mybir
import concourse.tile as tile
from concourse.bass_utils import run_bass_kernel_spmd

F32 = mybir.dt.float32
BF16 = mybir.dt.bfloat16
I16 = mybir.dt.int16
BF = ml_dtypes.bfloat16
AF = mybir.ActivationFunctionType
OP = mybir.AluOpType

P = 8          # cores
HEADS = 8
NEG_SLOPE = 0.2
NUM_Q = int(os.environ.get("GNN_NUM_Q", "4"))  # SWDGE queues for gathers
USE_PBCAST = os.environ.get("GNN_PBCAST", "1") == "1"

# per-layer: row elems (gather granule, 256B-multiple), C = value cols,
# es at cols C..C+8 for GAT layers.
LAYERS = [
    dict(row=384, C=256, ch=32, gat=True),
    dict(row=128, C=128, ch=None, gat=False),
    dict(row=640, C=512, ch=64, gat=True),
    dict(row=128, C=64, ch=None, gat=False),
]


def _wrap_idx(idx, cap):
    """dma_gather idx layout: [128, cap/16] i16; idx i at (i%16, i//16),
    replicated 8x down partitions. Positions >= len(idx) gather row 0."""
    n = len(idx)
    cols = cap // 16
    arr = np.zeros((16, cols), np.int16)
    for off in range(16):
        sub = idx[off::16]
        arr[off, : len(sub)] = sub
    return np.tile(arr, (8, 1))


def preprocess(edge_index, N):
    """Graph partitioning + per-core index metadata with local-row-half
    tables. Returns (sched, percore)."""
    NSH = N // P
    NB = (NSH + 127) // 128
    NBA = min((NB + 1) // 2, NB)
    HA = min(NSH, NBA * 128)          # rows in half A (blocks 0..NBA-1)
    HB = NSH - HA

    src = np.concatenate([np.asarray(edge_index[0]), np.arange(N)]).astype(np.int64)
    dst = np.concatenate([np.asarray(edge_index[1]), np.arange(N)]).astype(np.int64)
    deg = np.bincount(dst, minlength=N).astype(np.float64)
    dis = (1.0 / np.sqrt(np.maximum(deg, 1e-12))).astype(np.float32)

    core = dst // NSH
    blk = (dst % NSH) // 128
    rsrc = src % NSH
    csrc = src // NSH
    hi = (rsrc >= HA).astype(np.int64)
    gidx = np.where(hi == 0, csrc * HA + rsrc, csrc * HB + (rsrc - HA))

    counts = np.zeros((P, NB, 2), np.int64)
    np.add.at(counts, (core, blk, hi), 1)
    K = np.maximum(1, -(-counts // 128)).max(axis=0)  # [NB, 2] chunks per half

    order = np.lexsort((hi, blk, core))
    so_g, so_dst, so_core, so_blk, so_hi = (a[order] for a in (gidx, dst, core, blk, hi))

    tot_ch = int(K.sum())
    percore = []
    for c in range(P):
        m = so_core == c
        cg, cdst = so_g[m], so_dst[m]
        cblk, chi = so_blk[m], so_hi[m]
        sidx = np.zeros(tot_ch * 128, np.int64)
        dloc = np.full((tot_ch * 128,), 200.0, np.float32)
        pos = 0
        ptr = 0
        sw = []
        for b in range(NB):
            for h in range(2):
                n_e = int(counts[c, b, h])
                cap = int(K[b, h]) * 128
                sidx[pos: pos + n_e] = cg[ptr: ptr + n_e]
                dloc[pos: pos + n_e] = (cdst[ptr: ptr + n_e] - c * NSH - b * 128).astype(np.float32)
                ptr += n_e
                sw.append(_wrap_idx(sidx[pos: pos + n_e], cap))
                pos += cap
        assert ptr == m.sum()
        percore.append(dict(
            sidx=np.concatenate(sw, axis=1).astype(np.int16),
            dstl=dloc.reshape(tot_ch, 128).T.astype(BF).copy(),    # [128, tot_ch]
            dstlf=dloc.reshape(tot_ch, 128).astype(BF).copy(),     # [tot_ch, 128]
            dis=dis[c * NSH: (c + 1) * NSH],
        ))
    sched = dict(N=N, NSH=NSH, NB=NB, NBA=NBA, HA=HA, HB=HB, K=K, tot_ch=tot_ch)
    return sched, percore


def prep_weights(w, N):
    """Host-side weight packing (bf16) shared by all cores."""
    def b16(a):
        return np.asarray(a, np.float32).astype(BF)

    W1 = np.asarray(w["g1_W"], np.float32)
    ws1s = np.einsum("khj,hj->kh", W1.reshape(1024, 8, 32), np.asarray(w["g1_as"], np.float32))
    ws1d = np.einsum("khj,hj->kh", W1.reshape(1024, 8, 32), np.asarray(w["g1_ad"], np.float32))
    w1cat = np.concatenate([W1, np.asarray(w["m1_W"], np.float32), ws1s, ws1d], axis=1)  # [1024, 528]
    w1t = b16(w1cat).reshape(8, 128, 528).transpose(1, 0, 2).copy()  # [128, 8, 528]

    w2cat = np.concatenate([np.asarray(w["g2_W"], np.float32), np.asarray(w["m2_W"], np.float32)], axis=1)
    w2t = b16(w2cat).reshape(2, 128, 256).transpose(1, 0, 2).copy()  # [128, 2, 256]

    W3 = np.asarray(w["g3_W"], np.float32)
    ws3s = np.einsum("khj,hj->kh", W3.reshape(128, 8, 64), np.asarray(w["g3_as"], np.float32))
    ws3d = np.einsum("khj,hj->kh", W3.reshape(128, 8, 64), np.asarray(w["g3_ad"], np.float32))
    w3t = b16(np.concatenate([W3, np.asarray(w["m3_W"], np.float32), ws3s, ws3d], axis=1))  # [128, 592]

    w4t = b16(np.concatenate([np.asarray(w["g4_W"], np.float32), np.asarray(w["m4_W"], np.float32)], axis=1))  # [64, 4]

    def rep(v):
        return np.tile(np.asarray(v, np.float32)[None, :], (128, 1)).copy()

    return dict(
        w1=w1t, w2=w2t, w3=w3t, w4=w4t,
        bias1=rep(np.asarray(w["g1_b"]) + np.asarray(w["m1_b"])),
        bias2=rep(np.asarray(w["g2_b"]) + np.asarray(w["m2_b"])),
        bias3=rep(np.asarray(w["g3_b"]) + np.asarray(w["m3_b"])),
        bias4=rep(np.asarray(w["g4_b"]) + np.asarray(w["m4_b"])),
        iota=np.tile(np.arange(128, dtype=np.float32)[None, :], (128, 1)).astype(BF),
        iotap=np.arange(128, dtype=np.float32)[:, None].astype(BF).copy(),
        ident=np.eye(128, dtype=np.float32).astype(BF),
        onesr=np.ones((1, 128), dtype=np.float32).astype(BF),
        b1r=np.asarray(w["g1_b"] + w["m1_b"], np.float32)[None, :].astype(BF),
        b2r=np.asarray(w["g2_b"] + w["m2_b"], np.float32)[None, :].astype(BF),
        b3r=np.asarray(w["g3_b"] + w["m3_b"], np.float32)[None, :].astype(BF),
    )


def bcast(ap, count):
    """Append a 0-stride broadcast dim of `count` to an AP."""
    return bass.AP(ap.tensor, ap.offset, list(ap.ap) + [[0, count]])


def mid_bcast(ap, count):
    """Insert a 0-stride dim of `count` after the partition dim."""
    a = list(ap.ap)
    return bass.AP(ap.tensor, ap.offset, [a[0], [0, count]] + a[1:])


def flat_view(t_ap, c0, rowstride, a, b, nch=None):
    """View chunk-major flat tile [128, W] as rows: [128, nch, b-a] starting
    at chunk c0, row stride `rowstride`, cols [a, b)."""
    p0 = t_ap.ap[0]
    if nch is None:
        return bass.AP(t_ap.tensor, t_ap.offset + c0 * rowstride + a,
                       [p0, [1, b - a]])
    return bass.AP(t_ap.tensor, t_ap.offset + c0 * rowstride + a,
                   [p0, [rowstride, nch], [1, b - a]])


def build_nc(sched):
    N, NSH, NB, NBA = sched["N"], sched["NSH"], sched["NB"], sched["NBA"]
    HA, HB, K, tot_ch = sched["HA"], sched["HB"], sched["K"], sched["tot_ch"]
    KA = int(K[:, 0].max())
    KB = int(K[:, 1].max())
    KT = int((K[:, 0] + K[:, 1]).max())
    ROWMAX = max(c["row"] for c in LAYERS)

    nc = bacc.Bacc("TRN2", target_bir_lowering=False, debug=False, num_devices=P,
                   num_swdge_queues=NUM_Q)

    # ---- I/O ----
    xT_in = nc.dram_tensor("xT", [NB, 128, 1024], BF16, kind="ExternalInput")
    w1_in = nc.dram_tensor("w1", [128, 8, 528], BF16, kind="ExternalInput")
    w2_in = nc.dram_tensor("w2", [128, 2, 256], BF16, kind="ExternalInput")
    w3_in = nc.dram_tensor("w3", [128, 592], BF16, kind="ExternalInput")
    w4_in = nc.dram_tensor("w4", [64, 4], BF16, kind="ExternalInput")
    b1_in = nc.dram_tensor("bias1", [128, 256], F32, kind="ExternalInput")
    b2_in = nc.dram_tensor("bias2", [128, 128], F32, kind="ExternalInput")
    b3_in = nc.dram_tensor("bias3", [128, 64], F32, kind="ExternalInput")
    b4_in = nc.dram_tensor("bias4", [128, 2], F32, kind="ExternalInput")
    iota_in = nc.dram_tensor("iota", [128, 128], BF16, kind="ExternalInput")
    iotap_in = nc.dram_tensor("iotap", [128, 1], BF16, kind="ExternalInput")
    id_in = nc.dram_tensor("ident", [128, 128], BF16, kind="ExternalInput")
    onesr_in = nc.dram_tensor("onesr", [1, 128], BF16, kind="ExternalInput")
    b1r_in = nc.dram_tensor("b1r", [1, 256], BF16, kind="ExternalInput")
    b2r_in = nc.dram_tensor("b2r", [1, 128], BF16, kind="ExternalInput")
    b3r_in = nc.dram_tensor("b3r", [1, 64], BF16, kind="ExternalInput")
    dis_in = nc.dram_tensor("dis", [128, NB], F32, kind="ExternalInput")
    sidx_in = nc.dram_tensor("sidx", [128, tot_ch * 8], I16, kind="ExternalInput")
    dstl_in = nc.dram_tensor("dstl", [128, tot_ch], BF16, kind="ExternalInput")
    dstlf_in = nc.dram_tensor("dstlf", [tot_ch, 128], BF16, kind="ExternalInput")
    out_dram = nc.dram_tensor("out", [NSH, 2], F32, kind="ExternalOutput")

    # ---- internal DRAM: bounce shards (two halves) + allgathered tables ----
    mlp0_dram = nc.dram_tensor("mlp0d", [NB, 128, 256], BF16)
    rows = [c["row"] for c in LAYERS]
    TbA = [nc.dram_tensor(f"T{l+1}bA", [HA, rows[l]], BF16) for l in range(4)]
    TbB = [nc.dram_tensor(f"T{l+1}bB", [HB, rows[l]], BF16) for l in range(4)]
    TgA = [nc.dram_tensor(f"T{l+1}A", [P * HA, rows[l]], BF16, addr_space="Shared")
           for l in range(4)]
    TgB = [nc.dram_tensor(f"T{l+1}B", [P * HB, rows[l]], BF16, addr_space="Shared")
           for l in range(4)]

    blk_rows = [min(128, NSH - b * 128) for b in range(NB)]
    koff = np.concatenate([[0], np.cumsum(K.sum(axis=1))]).astype(int)

    gq = [0]  # gather queue round-robin
    pref = {}  # per-block prefetch stash

    def next_q():
        q = gq[0]
        gq[0] = (q + 1) % NUM_Q
        return q

    with tile.TileContext(nc) as tc:
        with (
            tc.tile_pool(name="consts", bufs=1) as cpool,
            tc.tile_pool(name="resident", bufs=1) as rpool,
            tc.tile_pool(name="meta", bufs=1) as mpool,
            tc.tile_pool(name="xload", bufs=2) as xpool,
            tc.tile_pool(name="gath", bufs=4) as gpool,
            tc.tile_pool(name="onehot", bufs=2) as opool,
            tc.tile_pool(name="dstlp", bufs=1) as dpool,
            tc.tile_pool(name="work", bufs=2) as wpool,
            tc.tile_pool(name="asm", bufs=2) as apool,
            tc.tile_pool(name="psA", bufs=2, space="PSUM") as psA,
            tc.tile_pool(name="psS", bufs=2, space="PSUM") as psS,
            tc.tile_pool(name="psT", bufs=2, space="PSUM") as psT,
            tc.tile_pool(name="psE", bufs=2, space="PSUM") as psE,
        ):
            # ---------- constants ----------
            w1_t = cpool.tile([128, 8, 528], BF16); nc.sync.dma_start(w1_t[:], w1_in[:])
            w2_t = cpool.tile([128, 2, 256], BF16); nc.sync.dma_start(w2_t[:], w2_in[:])
            w3_t = cpool.tile([128, 592], BF16); nc.sync.dma_start(w3_t[:], w3_in[:])
            w4_t = cpool.tile([64, 4], BF16); nc.sync.dma_start(w4_t[:], w4_in[:])
            b1_t = cpool.tile([128, 256], F32); nc.sync.dma_start(b1_t[:], b1_in[:])
            b2_t = cpool.tile([128, 128], F32); nc.sync.dma_start(b2_t[:], b2_in[:])
            b3_t = cpool.tile([128, 64], F32); nc.sync.dma_start(b3_t[:], b3_in[:])
            b4_t = cpool.tile([128, 2], F32); nc.sync.dma_start(b4_t[:], b4_in[:])
            iota_t = cpool.tile([128, 128], BF16); nc.sync.dma_start(iota_t[:], iota_in[:])
            iop_t = cpool.tile([128, 1], BF16); nc.sync.dma_start(iop_t[:], iotap_in[:])
            id_t = cpool.tile([128, 128], BF16); nc.sync.dma_start(id_t[:], id_in[:])
            ones_t = cpool.tile([1, 128], BF16); nc.sync.dma_start(ones_t[:], onesr_in[:])
            b1r_t = cpool.tile([1, 256], BF16); nc.sync.dma_start(b1r_t[:], b1r_in[:])
            b2r_t = cpool.tile([1, 128], BF16); nc.sync.dma_start(b2r_t[:], b2r_in[:])
            b3r_t = cpool.tile([1, 64], BF16); nc.sync.dma_start(b3r_t[:], b3r_in[:])
            dis_t = cpool.tile([128, NB], F32); nc.sync.dma_start(dis_t[:], dis_in[:])
            zcol = cpool.tile([128, 1], F32); nc.vector.memset(zcol[:], 0.0)
            epscol = cpool.tile([128, 1], F32); nc.vector.memset(epscol[:], 1e-5)
            sidx_t = mpool.tile([128, tot_ch * 8], I16); nc.sync.dma_start(sidx_t[:], sidx_in[:])
            dstl_t = mpool.tile([128, tot_ch], BF16); nc.sync.dma_start(dstl_t[:], dstl_in[:])

            mlp_sb = [None] + [rpool.tile([128, NB, c], BF16, name=f"mlp{i}_sb", tag=f"mlp{i}_sb")
                               for i, c in enumerate((128, 64), start=1)]
            x3_sb = rpool.tile([128, NB, 64], BF16)
            edl1 = rpool.tile([128, NB, 8], BF16)
            edl3 = rpool.tile([128, NB, 8], BF16)

            def tb_write(l, b, src_ap):
                r = blk_rows[b]
                if b < NBA:
                    nc.sync.dma_start(TbA[l][b * 128: b * 128 + r, :], src_ap[0:r, :])
                else:
                    r0 = b * 128 - HA
                    nc.sync.dma_start(TbB[l][r0: r0 + r, :], src_ap[0:r, :])

            def ag(l, half):
                src, dst = (TbA[l], TgA[l]) if half == 0 else (TbB[l], TgB[l])
                nc.gpsimd.collective_compute(
                    "AllGather", OP.bypass, replica_groups=[list(range(P))],
                    ins=[src.ap().opt()], outs=[dst.ap().opt()])

            # B-half blocks first so the B AllGather is emitted early and its
            # flight hides under the A-half processing of the same phase.
            order_blocks = list(range(NBA, NB)) + list(range(NBA))

            # ---------- dense phase 1: h1|mlp1|es1|ed1 from x ----------
            for b in order_blocks:
                xt = xpool.tile([128, 1024], BF16)
                nc.sync.dma_start(xt[:], xT_in[b])
                pd = psA.tile([128, 512], F32, tag="psA")
                pe = psS.tile([128, 16], F32, tag="psS")
                for k in range(8):
                    nc.tensor.matmul(pd[:], xt[:, k * 128:(k + 1) * 128],
                                     w1_t[:, k, 0:512], start=(k == 0), stop=(k == 7))
                nc.tensor.matmul(pd[:, 256:512], ones_t[:], b1r_t[:],
                                 start=False, stop=True, skip_group_check=True)
                for k in range(8):
                    nc.tensor.matmul(pe[:], xt[:, k * 128:(k + 1) * 128],
                                     w1_t[:, k, 512:528], start=(k == 0), stop=(k == 7))
                as1 = apool.tile([128, 384], BF16, tag="as1")
                nc.scalar.copy(as1[:, 0:256], pd[:, 0:256])
                nc.scalar.copy(as1[:, 256:264], pe[:, 0:8])
                tb_write(0, b, as1)
                nc.scalar.copy(edl1[:, b, :], pe[:, 8:16])
                m0 = apool.tile([128, 256], BF16, tag="m0")
                nc.scalar.copy(m0[:], pd[:, 256:512])
                nc.sync.dma_start(mlp0_dram[b], m0[:])
                if b == NB - 1:
                    ag(0, 1)
            ag(0, 0)

            # one-time zero of gather buffers (stale rows of padded gather
            # slots are read by DVE/PE; first use must not be NaN garbage)
            for _ in range(4):
                g = gpool.tile([128, KT * ROWMAX], BF16, tag="G")
                nc.vector.memset(g[:], 0.0)

            # ---------- propagation ----------
            def prop(l, epilogue):
                cfg = LAYERS[l - 1]
                row, C, gat = cfg["row"], cfg["C"], cfg["gat"]
                edl = edl1 if l == 1 else (edl3 if l == 3 else None)
                dstlP_slice = {}  # block -> (tile, chunk0) for paired pbcast
                for bi, b in enumerate(order_blocks):
                    ka, kb = int(K[b, 0]), int(K[b, 1])
                    kt = ka + kb
                    o = int(koff[b])
                    corder = list(range(ka, kt)) + list(range(ka))
                    g = gpool.tile([128, KT * ROWMAX], BF16, tag="G")
                    nc.gpsimd.dma_gather(
                        out_ap=flat_view(g[:], ka, row, 0, row, kb),
                        in_ap=TgB[l - 1][:],
                        idxs_ap=sidx_t[:, (o + ka) * 8: (o + kt) * 8],
                        num_idxs=kb * 128, num_idxs_reg=kb * 128, elem_size=row,
                        single_packet=False, queue_num=next_q())
                    nc.gpsimd.dma_gather(
                        out_ap=flat_view(g[:], 0, row, 0, row, ka),
                        in_ap=TgA[l - 1][:],
                        idxs_ap=sidx_t[:, o * 8: (o + ka) * 8],
                        num_idxs=ka * 128, num_idxs_reg=ka * 128, elem_size=row,
                        single_packet=False, queue_num=next_q())

                    def gview(c, a2, b2, n=None):
                        """row view of chunk c (global chunk id in block)."""
                        return flat_view(g[:], c, row, a2, b2, n)

                    if l == 1:
                        m0p = wpool.tile([128, 256], BF16, tag="m0l")
                        nc.sync.dma_start(m0p[:], mlp0_dram[b])
                        pref["m0"] = m0p

                    # one-hot [e, (k, d)] = (iota[d] == dstl[k][e])
                    oh = opool.tile([128, KT * 128], BF16, tag="oh")
                    ohv = bass.AP(oh[:].tensor, oh[:].offset,
                                  [oh[:].ap[0], [128, kt], [1, 128]])
                    nc.vector.tensor_tensor(
                        ohv, mid_bcast(iota_t[:], kt),
                        bcast(dstl_t[:, o: o + kt], 128), OP.is_equal)

                    pagg = psA.tile([128, 264] if gat else [128, C], F32, tag="psA")
                    if gat:
                        # transposed one-hot from a partition-broadcast of
                        # dst_local (for the per-edge ed fetch)
                        if b not in dstlP_slice:
                            blks = [b]
                            if bi + 1 < len(order_blocks) and order_blocks[bi + 1] == b + 1:
                                blks.append(b + 1)
                            k0 = int(K[blks[0]].sum())
                            ktot = sum(int(K[x].sum()) for x in blks)
                            op0 = int(koff[blks[0]])
                            dstlP = dpool.tile([128, 2 * KT * 128], BF16, tag="dstlP")
                            nc.sync.dma_start(dstlP[0:1, 0: ktot * 128],
                                              dstlf_in[op0: op0 + ktot, :])
                            nc.gpsimd.partition_broadcast(
                                dstlP[:, 0: ktot * 128], dstlP[0:1, 0: ktot * 128])
                            dstlP_slice[blks[0]] = (dstlP, 0)
                            if len(blks) > 1:
                                dstlP_slice[blks[1]] = (dstlP, k0)
                        dstlP, c0 = dstlP_slice.pop(b)
                        ohT = opool.tile([128, KT * 128], BF16, tag="ohT")
                        dPv = bass.AP(dstlP[:].tensor, dstlP[:].offset + c0 * 128,
                                      [dstlP[:].ap[0], [128, kt], [1, 128]])
                        iopv = bass.AP(iop_t[:].tensor, iop_t[:].offset,
                                       [iop_t[:].ap[0], [0, kt], [0, 128]])
                        ohTv = bass.AP(ohT[:].tensor, ohT[:].offset,
                                       [ohT[:].ap[0], [128, kt], [1, 128]])
                        nc.vector.tensor_tensor(ohTv, dPv, iopv, OP.is_equal)
                        pse = psE.tile([128, KT, 8], F32, tag="psE")
                        for c in range(kt):
                            nc.tensor.matmul(pse[:, c, :],
                                             bass.AP(ohT[:].tensor, ohT[:].offset + c * 128,
                                                     [ohT[:].ap[0], [1, 128]]),
                                             edl[:, b, :], start=True, stop=True,
                                             skip_group_check=True)
                        # logits -> exp weights: exp(lrelu(z)) = max(e^z, e^.2z)
                        tsum = wpool.tile([128, KT, 8], BF16, tag="tsum")
                        nc.vector.tensor_tensor(tsum[:, 0:kt, :], gview(0, C, C + 8, kt),
                                                pse[:, 0:kt, :], OP.add)
                        e1 = wpool.tile([128, KT, 8], BF16, tag="e1")
                        nc.scalar.activation(e1[:, 0:kt, :], tsum[:, 0:kt, :],
                                             AF.Exp, bias=zcol[:])
                        e5 = wpool.tile([128, KT, 8], BF16, tag="e5")
                        nc.scalar.activation(e5[:, 0:kt, :], tsum[:, 0:kt, :],
                                             AF.Exp, bias=zcol[:], scale=NEG_SLOPE)
                        nc.vector.tensor_tensor(gview(0, C, C + 8, kt), e1[:, 0:kt, :],
                                                e5[:, 0:kt, :], OP.max)
                        # sc = h * exp (in place, head-broadcast)
                        ch = cfg["ch"]
                        hview = gview(0, 0, C, kt)
                        h4 = bass.AP(hview.tensor, hview.offset,
                                     [hview.ap[0], [row, kt], [ch, HEADS], [1, ch]])
                        ex = gview(0, C, C + 8, kt)
                        exb = bass.AP(ex.tensor, ex.offset,
                                      [ex.ap[0], [row, kt], [1, HEADS], [0, ch]])
                        nc.vector.tensor_tensor(h4, h4, exb, OP.mult)
                        # fused aggregate + denominator matmuls
                        if C == 256:
                            for i, c in enumerate(corder):
                                nc.tensor.matmul(pagg[:], ohv[:, c, :], gview(c, 0, 264),
                                                 start=(i == 0), stop=(i == kt - 1),
                                                 skip_group_check=True)
                            den = pagg[:, 256:264]
                        else:
                            pag2 = psA.tile([128, 264], F32, tag="psA")
                            for i, c in enumerate(corder):
                                nc.tensor.matmul(pagg[:, 0:256], ohv[:, c, :], gview(c, 0, 256),
                                                 start=(i == 0), stop=(i == kt - 1),
                                                 skip_group_check=True)
                            for i, c in enumerate(corder):
                                nc.tensor.matmul(pag2[:], ohv[:, c, :], gview(c, 256, 520),
                                                 start=(i == 0), stop=(i == kt - 1),
                                                 skip_group_check=True)
                            den = pag2[:, 256:264]
                        sden = wpool.tile([128, 8], F32, tag="sden")
                        nc.vector.tensor_scalar(sden[:], den, 1e-16, None, OP.add)
                        rs = wpool.tile([128, 8], F32, tag="rs")
                        nc.vector.reciprocal(rs[:], sden[:])
                        agf = wpool.tile([128, C], F32, tag="agf")
                        if C == 256:
                            nc.vector.tensor_tensor(
                                agf[:].rearrange("p (h j) -> p h j", j=ch),
                                pagg[:, 0:256].rearrange("p (h j) -> p h j", j=ch),
                                bcast(rs[:], ch), OP.mult)
                        else:
                            nc.vector.tensor_tensor(
                                agf[:, 0:256].rearrange("p (h j) -> p h j", j=ch),
                                pagg[:, 0:256].rearrange("p (h j) -> p h j", j=ch),
                                bcast(rs[:, 0:4], ch), OP.mult)
                            nc.vector.tensor_tensor(
                                agf[:, 256:512].rearrange("p (h j) -> p h j", j=ch),
                                pag2[:, 0:256].rearrange("p (h j) -> p h j", j=ch),
                                bcast(rs[:, 4:8], ch), OP.mult)
                        epilogue(b, agf)
                    else:
                        for i, c in enumerate(corder):
                            nc.tensor.matmul(pagg[:], ohv[:, c, :], gview(c, 0, C),
                                             start=(i == 0), stop=(i == kt - 1),
                                             skip_group_check=True)
                        agf = wpool.tile([128, C], F32, tag="agf")
                        nc.scalar.mul(agf[:], pagg[:], dis_t[:, b: b + 1])
                        epilogue(b, agf)
                    if b == NB - 1 and l < 4:
                        ag(l, 1)
                if l < 4:
                    ag(l, 0)

            def layer_norm(t, Cn):
                """LN over free dim (ln weight=1 bias=0); scalar-engine heavy.
                In-place: t is centered and scaled; returns bf16 tile."""
                mu = wpool.tile([128, 1], F32, tag="mu")
                nc.vector.tensor_reduce(mu[:], t[:], mybir.AxisListType.X, OP.add)
                mun = wpool.tile([128, 1], F32, tag="mun")
                nc.scalar.mul(mun[:], mu[:], -1.0 / Cn)
                nc.scalar.activation(t[:], t[:], AF.Identity, bias=mun[:])
                sq = wpool.tile([128, Cn], BF16, tag="sq")
                vs = wpool.tile([128, 1], F32, tag="vs")
                nc.scalar.activation(sq[:], t[:], AF.Square, bias=zcol[:], accum_out=vs[:])
                sd = wpool.tile([128, 1], F32, tag="sd")
                nc.scalar.activation(sd[:], vs[:], AF.Sqrt, bias=epscol[:], scale=1.0 / Cn)
                rstd = wpool.tile([128, 1], F32, tag="rstd")
                nc.vector.reciprocal(rstd[:], sd[:])
                xo = wpool.tile([128, Cn], BF16, tag="xo")
                nc.scalar.mul(xo[:], t[:], rstd[:])
                return xo

            def transpose_to(x_ap, cols):
                pt = psT.tile([128, 128], BF16, tag="psT")
                nc.tensor.transpose(pt[0:cols, :], x_ap, id_t[:])
                xt_ = wpool.tile([cols, 128], BF16, tag=f"tr{cols}")
                nc.scalar.copy(xt_[:], pt[0:cols, :])
                return xt_

            # ---- L1 epilogue: LN -> x1, dense-2 (h2'|mlp2), T2 assembly ----
            def epi1(b, agf):
                nc.vector.tensor_tensor(agf[:], agf[:], pref["m0"][:], OP.add)
                x1 = layer_norm(agf, 256)
                xta = transpose_to(x1[:, 0:128], 128)
                xtb = transpose_to(x1[:, 128:256], 128)
                ps2 = psS.tile([128, 256], F32, tag="psS")
                nc.tensor.matmul(ps2[:], xta[:], w2_t[:, 0, :], start=True, stop=False)
                nc.tensor.matmul(ps2[:], xtb[:], w2_t[:, 1, :], start=False, stop=True)
                nc.tensor.matmul(ps2[:, 128:256], ones_t[:], b2r_t[:],
                                 start=False, stop=True, skip_group_check=True)
                as2 = apool.tile([128, 128], BF16, tag="as2")
                nc.scalar.mul(as2[:], ps2[:, 0:128], dis_t[:, b: b + 1])
                tb_write(1, b, as2)
                nc.scalar.copy(mlp_sb[1][:, b, :], ps2[:, 128:256])

            # ---- L2 epilogue: LN -> x2, dense-3, T3 assembly ----
            def epi2(b, agf):
                nc.vector.tensor_tensor(agf[:], agf[:], mlp_sb[1][:, b, :], OP.add)
                x2 = layer_norm(agf, 128)
                xt2 = transpose_to(x2[:], 128)
                ps3a = psA.tile([128, 512], F32, tag="psA")
                nc.tensor.matmul(ps3a[:], xt2[:], w3_t[:, 0:512], start=True, stop=True)
                ps3b = psS.tile([128, 80], F32, tag="psS")
                nc.tensor.matmul(ps3b[:], xt2[:], w3_t[:, 512:592], start=True, stop=False)
                nc.tensor.matmul(ps3b[:, 0:64], ones_t[:], b3r_t[:],
                                 start=False, stop=True, skip_group_check=True)
                as3 = apool.tile([128, 640], BF16, tag="as3")
                nc.scalar.copy(as3[:, 0:512], ps3a[:])
                nc.scalar.copy(as3[:, 512:520], ps3b[:, 64:72])
                tb_write(2, b, as3)
                nc.scalar.copy(edl3[:, b, :], ps3b[:, 72:80])
                nc.scalar.copy(mlp_sb[2][:, b, :], ps3b[:, 0:64])

            # ---- L3 epilogue: mean heads, LN -> x3, T4 assembly ----
            def epi3(b, agf):
                mf = wpool.tile([128, 64], F32, tag="mf")
                a = agf[:]
                nc.vector.tensor_reduce(
                    mf[:], bass.AP(a.tensor, a.offset, [a.ap[0], [1, 64], [64, 8]]),
                    mybir.AxisListType.X, OP.add)
                t = wpool.tile([128, 64], F32, tag="t3")
                nc.scalar.mul(t[:], mf[:], 0.125)
                nc.vector.tensor_tensor(t[:], t[:], mlp_sb[2][:, b, :], OP.add)
                x3 = layer_norm(t, 64)
                nc.scalar.copy(x3_sb[:, b, :], x3[:])
                as4 = apool.tile([128, 128], BF16, tag="as4")
                nc.scalar.mul(as4[:, 0:64], x3[:], dis_t[:, b: b + 1])
                tb_write(3, b, as4)

            # ---- L4 epilogue: (agg @ W4) + (x3 @ m4_W) + bias ----
            def epi4(b, agf):
                a4 = wpool.tile([128, 64], BF16, tag="a4")
                nc.scalar.copy(a4[:], agf[:])
                a4T = transpose_to(a4[:], 64)
                x3T = transpose_to(x3_sb[:, b, :], 64)
                ps4 = psS.tile([128, 2], F32, tag="psS")
                nc.tensor.matmul(ps4[:], a4T[:], w4_t[:, 0:2], start=True, stop=False)
                nc.tensor.matmul(ps4[:], x3T[:], w4_t[:, 2:4], start=False, stop=True)
                ot = wpool.tile([128, 2], F32, tag="ot")
                nc.vector.tensor_tensor(ot[:], ps4[:], b4_t[:], OP.add)
                nc.sync.dma_start(out_dram[b * 128: b * 128 + blk_rows[b], :], ot[0:blk_rows[b], :])

            prop(1, epi1)
            prop(2, epi2)
            prop(3, epi3)
            prop(4, epi4)

    nc.compile()
    return nc


def make_in_maps(inputs, sched, percore):
    N, NSH, NB = sched["N"], sched["NSH"], sched["NB"]
    wm = prep_weights(inputs, N)
    x = np.asarray(inputs["x"], np.float32)
    in_maps = []
    for c in range(P):
        xs = x[c * NSH: (c + 1) * NSH]
        pad = NB * 128 - NSH
        if pad:
            xs = np.concatenate([xs, np.zeros((pad, 1024), np.float32)], 0)
        xT = xs.astype(BF).reshape(NB, 128, 8, 128).transpose(0, 3, 2, 1).reshape(NB, 128, 1024).copy()
        pc = percore[c]
        dis = np.zeros((128, NB), np.float32)
        dv = pc["dis"]
        for b in range(NB):
            r = min(128, NSH - b * 128)
            dis[0:r, b] = dv[b * 128: b * 128 + r]
        in_maps.append(dict(
            xT=xT, w1=wm["w1"], w2=wm["w2"], w3=wm["w3"], w4=wm["w4"],
            bias1=wm["bias1"], bias2=wm["bias2"], bias3=wm["bias3"], bias4=wm["bias4"],
            iota=wm["iota"], iotap=wm["iotap"], ident=wm["ident"], dis=dis,
            onesr=wm["onesr"], b1r=wm["b1r"], b2r=wm["b2r"], b3r=wm["b3r"],
            sidx=pc["sidx"], dstl=pc["dstl"], dstlf=pc["dstlf"],
        ))
    return in_maps


def run(inputs, N=50000, trace=False):
    sched, percore = preprocess(np.asarray(inputs["edge_index"]), N)
    in_maps = make_in_maps(inputs, sched, percore)
    nc = build_nc(sched)
    res = run_bass_kernel_spmd(nc, in_maps, core_ids=list(range(P)), trace=trace)
    out = np.concatenate([res.results[c]["out"] for c in range(P)], axis=0)
    return out, res


def kernel(**inputs):
    out, _ = run(inputs, N=50000)
    return out.astype(np.float32)
